# revision 14
# baseline (speedup 1.0000x reference)
"""DigitCaps (capsule routing) Trainium2 kernel, v2 (no-maxshift design).

Self-contained: hardcodes shapes for
  x: [256, 32, 8, 6, 6] f32, W: [1, 10, 1152, 16, 8] f32 -> v: [256, 10, 16] f32

Sharding: pure data parallelism over batch, 32 batch items per core on 8
cores, processed as 4 octet groups per core.

Per-core layout: partition p = (i16, b8); u free dims ordered (ic=72,
w=16, h=10) with h innermost so the big DVE muls run in fp16 2x mode.
u = W@x from block-diag packed fp16 matmuls (K=128: lhsT = host-built
block-diag x tile, rhs = repacked W, w-major/h-minor free order).

v2 changes vs v1:
  - s0 = mean_i u_i computed directly as a dense K=9216 contraction
    (72 accumulating matmuls of xcont[ic] @ wpack[:, ic], out [32, 160]),
    not by streaming u through sdelta matmuls. xcont is pre-scaled 1/NI.
  - NO softmax max-shift. Iteration-1 logits are in [-4, 6] so exp fits
    f16 directly; iteration-2 logits reach ~36 so exp goes to bf16
    (range to 3e38) and pr = u*c is bf16. The softmax division happens in
    squash via the streamed denominator (PSUM-accumulated dps matmuls).
  - Unified [32]-partition s/d/squash layout: per-group sdelta32/srepl32
    selection matrices let all 4 groups accumulate into one PSUM tile;
    one reduce + one squash + one output DMA per iteration.
  - u PSUM->SBUF copies in chunks of 6 ic (fewer ACT instructions), some
    offloaded to the otherwise-idle gpsimd (Pool) engine.
  - a-pass fold over w either as in-place DVE fp16 fold tree or on PE
    (16 accumulating identity matmuls), chosen per group for balance.
"""

import numpy as np

# ---- problem constants (hardcoded) ----
B_FULL = 256
N_CORES = 8
B_CORE = B_FULL // N_CORES          # 32
NGRP = 4                            # octet groups per core
B8 = 8                              # batch per group
H = 10
WD = 16
WH = WD * H                         # 160
S = 8
NI = 1152
I16 = 16
IC = NI // I16                      # 72
ICQ = 9                             # ic per wpack DMA chunk
XDC = 18                            # ic per xdiag DMA chunk
CPY = 3                             # ic per u psum copy tile (1 PSUM bank)
CPYS = 3                            # ic per sps matmul
CKS = 36                            # ic per s-pass mul chunk
HIC = IC // 2                       # 36 (a-pass half)
P = 128

_CACHE = {}


def _build_program(debug: bool, dumps: bool = False):
    import concourse.bacc as bacc
    import concourse.bass as bass
    import concourse.tile as tile
    from concourse import mybir

    f32 = mybir.dt.float32
    f16 = mybir.dt.float16
    bf16 = mybir.dt.bfloat16
    AX = mybir.AxisListType
    AF = mybir.ActivationFunctionType

    if not getattr(bacc, "_digitcaps_act_pin", False):
        _orig_gat = bacc.get_activation_tables

        def _pinned_gat(arch):
            tables = dict(_orig_gat(arch))
            both = {mybir.ActivationFunctionType.Exp, mybir.ActivationFunctionType.Ln}
            for name in tables:
                if name != "natural_log_exp_and_others" and both & tables[name]:
                    tables[name] = tables[name] - both
            return tables

        bacc.get_activation_tables = _pinned_gat
        bacc._digitcaps_act_pin = True

    nc = bacc.Bacc(
        "TRN2", target_bir_lowering=False, debug=debug, enable_asserts=False
    )

    xd_d = nc.dram_tensor("xdiag", [NGRP, P, IC * P], f16, kind="ExternalInput")
    w_d = nc.dram_tensor("wpack", [P, IC * WH], f16, kind="ExternalInput")
    xc_d = nc.dram_tensor("xcont", [P, IC * B_CORE], f16, kind="ExternalInput")
    sd_d = nc.dram_tensor("sdel32", [P, NGRP * B_CORE], f16, kind="ExternalInput")
    sdb_d = nc.dram_tensor("sdel32b", [P, NGRP * B_CORE], bf16, kind="ExternalInput")
    sr_d = nc.dram_tensor("srep32", [B_CORE, NGRP * P], f16, kind="ExternalInput")
    id_d = nc.dram_tensor("ident", [P, P], f16, kind="ExternalInput")
    out_d = nc.dram_tensor("vout", [B_CORE, WH], f32, kind="ExternalOutput")
    if dumps:
        dbg_u = nc.dram_tensor("dbg_u", [P, IC * WH], f16, kind="ExternalOutput")
        dbg_l = nc.dram_tensor("dbg_l", [P, NGRP * IC * H], f16, kind="ExternalOutput")
        dbg_c = nc.dram_tensor("dbg_c", [P, NGRP * IC * H], f16, kind="ExternalOutput")
        dbg_s = nc.dram_tensor("dbg_s", [B_CORE, 170], f32, kind="ExternalOutput")
        dbg_s2 = nc.dram_tensor("dbg_s2", [B_CORE, 170], f32, kind="ExternalOutput")
        dbg_v = nc.dram_tensor("dbg_v", [B_CORE, WH], f16, kind="ExternalOutput")
        dbg_s0 = nc.dram_tensor("dbg_s0", [B_CORE, WH], f32, kind="ExternalOutput")

    with tile.TileContext(nc) as tc:
        with (
            tc.tile_pool(name="const", bufs=1) as const_pool,
            tc.tile_pool(name="wp", bufs=1) as wp_pool,
            tc.tile_pool(name="xd", bufs=2) as xd_pool,
            tc.tile_pool(name="u", bufs=4) as u_pool,
            tc.tile_pool(name="prs", bufs=2) as prs_pool,
            tc.tile_pool(name="pra", bufs=2) as pra_pool,
            tc.tile_pool(name="lg", bufs=1) as lg_pool,
            tc.tile_pool(name="cexp", bufs=1) as c_pool,
            tc.tile_pool(name="small", bufs=2) as small_pool,
            tc.tile_pool(name="vv", bufs=1) as vv_pool,
            tc.tile_pool(name="psum_u", bufs=2, space="PSUM") as psum_u,
            tc.tile_pool(name="psum_s", bufs=1, space="PSUM") as psum_s,
            tc.tile_pool(name="psum_v", bufs=2, space="PSUM") as psum_v,
        ):
            xcont = const_pool.tile([P, IC, B_CORE], f16, tag="xcont")
            nc.sync.dma_start(
                xcont[:], xc_d[:].rearrange("p (ic b) -> p ic b", ic=IC)
            )
            srep32 = const_pool.tile([B_CORE, NGRP, P], f16, tag="srep32")
            nc.sync.dma_start(
                srep32[:], sr_d[:].rearrange("b (g p) -> b g p", g=NGRP)
            )

            def load_xd(g, xc):
                # xdiag rides the Activation HWDGE queue so it streams in
                # parallel with wpack/xcont on the sync queue.
                xd = xd_pool.tile([P, XDC, P], f16, tag="xd")
                nc.scalar.dma_start(
                    xd[:],
                    xd_d[g].rearrange("p (ic m) -> p ic m", ic=IC)[
                        :, xc : xc + XDC
                    ],
                )
                return xd

            # resident W pack, split per chunk so deps are chunk-granular.
            # Emission interleaves group 0's xd loads between wpack chunks and
            # streams the 72 s0 matmuls as each W chunk arrives.
            wpq = [None] * (IC // ICQ)
            xds0 = []

            def load_wp(qi):
                wq = wp_pool.tile([P, ICQ, WH], f16, tag=f"wp{qi}")
                nc.sync.dma_start(
                    wq[:],
                    w_d[:].rearrange("p (ic f) -> p ic f", f=WH)[
                        :, qi * ICQ : (qi + 1) * ICQ
                    ],
                )
                wpq[qi] = wq

            # s0[b, (w,h)] = sum_{ic,i16,s} x[b,i,s]/NI * W[h,i,w,s]
            s0ps_t = psum_s.tile([B_CORE, CPYS, 170], f32, tag="sps")
            s0ps = s0ps_t[:, 0, 0:WH]

            def s0_chunk(qi):
                for t in range(ICQ):
                    ic = qi * ICQ + t
                    nc.tensor.matmul(
                        s0ps,
                        xcont[:, ic, :],
                        wpq[qi][:, t, :],
                        start=(ic == 0),
                        stop=(ic == IC - 1),
                    )

            load_wp(0)
            xds0.append(load_xd(0, 0))
            s0_chunk(0)
            load_wp(1)
            s0_chunk(1)
            load_wp(2)
            xds0.append(load_xd(0, XDC))
            s0_chunk(2)
            load_wp(3)
            s0_chunk(3)
            load_wp(4)
            xds0.append(load_xd(0, 2 * XDC))
            s0_chunk(4)
            load_wp(5)
            s0_chunk(5)
            load_wp(6)
            xds0.append(load_xd(0, 3 * XDC))
            s0_chunk(6)
            load_wp(7)
            s0_chunk(7)

            # remaining consts (needed only from iteration 1 onward)
            sdel32 = const_pool.tile([P, NGRP, B_CORE], f16, tag="sdel32")
            nc.sync.dma_start(
                sdel32[:], sd_d[:].rearrange("p (g b) -> p g b", g=NGRP)
            )
            sdel32b = const_pool.tile([P, NGRP, B_CORE], bf16, tag="sdel32b")
            nc.sync.dma_start(
                sdel32b[:], sdb_d[:].rearrange("p (g b) -> p g b", g=NGRP)
            )
            ident = const_pool.tile([P, P], f16, tag="ident")
            nc.sync.dma_start(ident[:], id_d[:])

            # persistent logits [P, g, ic, h] f16
            logits = lg_pool.tile([P, NGRP, IC, H], f16, tag="logits")

            # V = running sum of v (f16) on 32 partitions; sun = s/d scratch
            V32 = vv_pool.tile([B_CORE, WD, H], f16, tag="V32")
            vb16 = vv_pool.tile([P, NGRP, WD, H], f16, tag="vb16")
            sun32 = vv_pool.tile([B_CORE, 170], f32, tag="sun32")

            us = []

            def ugen_part(g, u, xchunks):
                """u-gen for group g over the given xd chunk indices.
                PSUM tile is [P, 2, 512] f32 (two banks; each matmul's
                160-col slice stays within one bank) so one ACT instruction
                drains 6 ic at a time."""
                for xi in xchunks:
                    xc = xi * XDC
                    xd = xds0[xi] if g == 0 else load_xd(g, xc)
                    for j in range(0, XDC, 2 * CPY):
                        ps = psum_u.tile([P, 2, 512], f32, tag="ups")
                        for t in range(2 * CPY):
                            ic = xc + j + t
                            k, m = divmod(t, CPY)
                            nc.tensor.matmul(
                                ps[:, k, m * WH : (m + 1) * WH],
                                xd[:, j + t, :],
                                wpq[ic // ICQ][:, ic % ICQ, :],
                                start=True,
                                stop=True,
                            )
                        ic0 = xc + j
                        nc.scalar.copy(
                            u[:, ic0 : ic0 + 2 * CPY].rearrange(
                                "p (k a) w h -> p k a w h", k=2
                            ),
                            ps[:, :, 0 : CPY * WH].rearrange(
                                "p k (a w h) -> p k a w h", w=WD, h=H
                            ),
                        )

            def ugen(g):
                u = u_pool.tile([P, IC, WD, H], f16, tag="u")
                ugen_part(g, u, range(IC // XDC))
                us.append(u)

            def squash(it):
                """sun32 -> v; updates V32 (it<2) or returns vfin (it=2)."""
                sw = sun32[:, 0:WH].rearrange("b (w h) -> b w h", h=H)
                if it == 0:
                    s = sw
                else:
                    dinv = small_pool.tile([B_CORE, H], f32, tag="dinv")
                    nc.vector.reciprocal(dinv[:], sun32[:, WH:170])
                    st = small_pool.tile([B_CORE, WD, H], f32, tag="st")
                    nc.vector.tensor_mul(
                        st[:], sw, dinv[:].unsqueeze(1).to_broadcast([B_CORE, WD, H])
                    )
                    s = st[:]
                s2 = small_pool.tile([B_CORE, WD, H], f32, tag="s2")
                nc.scalar.activation(s2[:], s, AF.Square)
                sq = small_pool.tile([B_CORE, H], f32, tag="sq")
                nc.vector.reduce_sum(
                    sq[:], s2[:].rearrange("b w h -> b h w"), axis=AX.X
                )
                lgq = small_pool.tile([B_CORE, H], f32, tag="lgq")
                nc.scalar.activation(lgq[:], sq[:], AF.Ln)
                rt = small_pool.tile([B_CORE, H], f32, tag="rt")
                nc.scalar.activation(rt[:], lgq[:], AF.Exp, scale=0.5)
                onep = small_pool.tile([B_CORE, H], f32, tag="onep")
                nc.vector.tensor_scalar_add(onep[:], sq[:], 1.0)
                rr = small_pool.tile([B_CORE, H], f32, tag="rr")
                nc.vector.reciprocal(rr[:], onep[:])
                f = small_pool.tile([B_CORE, H], f32, tag="f")
                nc.vector.tensor_mul(f[:], rt[:], rr[:])
                fb = f[:].unsqueeze(1).to_broadcast([B_CORE, WD, H])
                if it == 2:
                    vfin = small_pool.tile([B_CORE, WD, H], f32, tag="vfin")
                    nc.vector.tensor_mul(vfin[:], s, fb)
                    return vfin
                if it == 0:
                    nc.vector.tensor_mul(V32[:], s, fb)
                else:
                    v16 = small_pool.tile([B_CORE, WD, H], f16, tag="v16")
                    nc.vector.tensor_mul(v16[:], s, fb)
                    nc.vector.tensor_add(V32[:], V32[:], v16[:])
                return None

            def vbcast():
                """vb16[:, g] = broadcast of V32 rows g*8..g*8+8."""
                for g in range(NGRP):
                    pv = psum_v.tile([P, HIC, H], f32, tag="pv")
                    vbp = pv[:, 0:WD, :].rearrange("p w h -> p w h")
                    nc.tensor.matmul(
                        vbp, srep32[:, g, :], V32[:], start=True, stop=True
                    )
                    nc.scalar.copy(vb16[:, g], vbp)

            def apass(g, pe_fold=False):
                """logits[:, g] = sum_w u * vb16[:, g], in two ic-halves."""
                u = us[g]
                for a in (0, HIC):
                    pra = pra_pool.tile([P, HIC, WD, H], f16, tag="pra")
                    vbb = vb16[:, g].unsqueeze(1).to_broadcast([P, HIC, WD, H])
                    nc.vector.tensor_mul(pra[:], u[:, a : a + HIC], vbb)
                    lslice = logits[:, g, a : a + HIC, :]
                    if pe_fold:
                        pa = psum_v.tile([P, HIC, H], f32, tag="pv")
                        for w in range(WD):
                            nc.tensor.matmul(
                                pa[:],
                                ident[:],
                                pra[:, :, w, :],
                                start=(w == 0),
                                stop=(w == WD - 1),
                            )
                        nc.scalar.copy(lslice, pa[:])
                    else:
                        nc.vector.tensor_add(
                            pra[:, :, 0:8, :],
                            pra[:, :, 0:8, :],
                            pra[:, :, 8:16, :],
                        )
                        nc.vector.tensor_add(
                            pra[:, :, 0:4, :],
                            pra[:, :, 0:4, :],
                            pra[:, :, 4:8, :],
                        )
                        nc.vector.tensor_add(
                            pra[:, :, 0:2, :],
                            pra[:, :, 0:2, :],
                            pra[:, :, 2:4, :],
                        )
                        nc.vector.tensor_add(
                            lslice, pra[:, :, 0, :], pra[:, :, 1, :]
                        )

            def spass(g, cexp, prdt, sdel, sps32, dps32):
                """stream d (dps) and s (sps) partial sums for group g."""
                # d: 2 matmuls over cexp halves
                for hi, a in enumerate((0, HIC)):
                    nc.tensor.matmul(
                        dps32[:],
                        sdel[:, g, :],
                        cexp[:, g, a : a + HIC, :],
                        start=(g == 0 and hi == 0),
                        stop=(g == NGRP - 1 and hi == 1),
                    )
                u = us[g]
                for c0 in range(0, IC, CKS):
                    pr = prs_pool.tile([P, CKS, WD, H], prdt, tag="pr")
                    cb = (
                        cexp[:, g, c0 : c0 + CKS, :]
                        .unsqueeze(2)
                        .to_broadcast([P, CKS, WD, H])
                    )
                    nc.vector.tensor_mul(pr[:], u[:, c0 : c0 + CKS], cb)
                    for j in range(0, CKS, CPYS):
                        ic = c0 + j
                        nc.tensor.matmul(
                            sps32[:, :, 0:WH],
                            sdel[:, g, :],
                            pr[:, j : j + CPYS],
                            start=(g == 0 and ic == 0),
                            stop=(g == NGRP - 1 and ic == IC - CPYS),
                        )

            def s_reduce(sps32, dps32):
                nc.vector.reduce_sum(
                    sun32[:, 0:WH],
                    sps32[:, :, 0:WH].rearrange("b a f -> b f a"),
                    axis=AX.X,
                )
                nc.vector.reduce_sum(
                    sun32[:, WH:170],
                    dps32[:].rearrange("b i h -> b h i"),
                    axis=AX.X,
                )

            # ======== iteration 0: s0 -> v0 -> vbcast -> apass ========
            # squash(0) first in emission so its ACT/DVE ops aren't queued
            # behind group-0's u copies; ugen(0) keeps PE busy meanwhile.
            # vbcast slots between ugen(0) halves so the PE in-order queue
            # reaches it as soon as V32 is ready (not after all of ugen).
            nc.scalar.copy(sun32[:, 0:WH], s0ps)
            if dumps:
                nc.sync.dma_start(dbg_s0[:], s0ps)
            squash(0)
            u0 = u_pool.tile([P, IC, WD, H], f16, tag="u")
            ugen_part(0, u0, range(2))
            vbcast()
            ugen_part(0, u0, range(2, IC // XDC))
            us.append(u0)
            apass(0, pe_fold=False)
            for g in range(1, NGRP):
                ugen(g)
                apass(g, pe_fold=(g >= 2))

            if dumps:
                nc.sync.dma_start(
                    dbg_u[:], us[0][:].rearrange("p ic w h -> p (ic w h)")
                )
                nc.sync.dma_start(
                    dbg_l[:], logits[:].rearrange("p g ic h -> p (g ic h)")
                )

            # ======== iteration 1 (f16 exp) ========
            cexp1 = c_pool.tile([P, NGRP, IC, H], f16, tag="cexp1")
            dps32 = psum_s.tile([B_CORE, HIC, H], f32, tag="dps")
            sps32 = psum_s.tile([B_CORE, CPYS, 170], f32, tag="sps")
            for g in range(NGRP):
                nc.scalar.activation(cexp1[:, g], logits[:, g], AF.Exp)
                spass(g, cexp1, f16, sdel32, sps32, dps32)
            if dumps:
                nc.sync.dma_start(
                    dbg_c[:], cexp1[:].rearrange("p g ic h -> p (g ic h)")
                )
            s_reduce(sps32, dps32)
            if dumps:
                nc.sync.dma_start(dbg_s[:], sun32[:])
            squash(1)
            if dumps:
                nc.sync.dma_start(
                    dbg_v[:], V32[:].rearrange("b w h -> b (w h)")
                )
            vbcast()
            for g in range(NGRP):
                apass(g, pe_fold=(g < NGRP - 1))

            # ======== iteration 2 (bf16 exp, bf16 pr) ========
            cexp2 = c_pool.tile([P, NGRP, IC, H], bf16, tag="cexp2")
            dps32b = psum_s.tile([B_CORE, HIC, H], f32, tag="dps")
            sps32b = psum_s.tile([B_CORE, CPYS, 170], f32, tag="sps")
            for g in range(NGRP):
                nc.scalar.activation(cexp2[:, g], logits[:, g], AF.Exp)
                spass(g, cexp2, bf16, sdel32b, sps32b, dps32b)
            s_reduce(sps32b, dps32b)
            if dumps:
                nc.sync.dma_start(dbg_s2[:], sun32[:])
            vfin = squash(2)
            nc.sync.dma_start(
                out_d[:], vfin[:].rearrange("b w h -> b (w h)")
            )

    nc.compile()
    return nc


def _host_inputs(x: np.ndarray, W: np.ndarray):
    """Build per-core input maps."""
    xr = np.ascontiguousarray(x.reshape(B_FULL, NI, S).astype(np.float32, copy=False))
    W0 = np.asarray(W, dtype=np.float32).reshape(H, NI, WD, S)
    # wpack[(i16,s), (ic, w, h)] = W0[h, ic*16+i16, w, s]
    wpack = np.ascontiguousarray(
        W0.reshape(H, IC, I16, WD, S)
        .transpose(2, 4, 1, 3, 0)
        .reshape(P, IC * WH)
        .astype(np.float16)
    )
    # sdel32[(i16,b8), (g, b32)] = (b32 == g*8 + b8)
    b8 = np.arange(P) % B8
    g_idx = np.arange(NGRP)
    b32 = np.arange(B_CORE)
    sdel = (
        b32[None, None, :] == (g_idx[None, :, None] * B8 + b8[:, None, None])
    ).astype(np.float16)
    sdel32 = np.ascontiguousarray(sdel.reshape(P, NGRP * B_CORE))
    import ml_dtypes

    sdel32b = sdel32.astype(ml_dtypes.bfloat16)
    # srep32[b32, (g, p)] = (b32 == g*8 + p%8)
    srep = (
        b32[:, None, None] == (g_idx[None, :, None] * B8 + b8[None, None, :])
    ).astype(np.float16)
    srep32 = np.ascontiguousarray(srep.reshape(B_CORE, NGRP * P))
    ident = np.eye(P, dtype=np.float16)

    in_maps = []
    for c in range(N_CORES):
        xc = xr[c * B_CORE : (c + 1) * B_CORE]  # [32, 1152, 8]
        # xdiag[g, (i16,s), ic*128 + i16*8 + b] = xc[g*8+b, ic*16+i16, s]
        xd = np.zeros((NGRP, P, IC, I16, B8), dtype=np.float16)
        xg = xc.reshape(NGRP, B8, IC, I16, S).astype(np.float16)
        for k in range(I16):
            xd[:, k * S : (k + 1) * S, :, k, :] = xg[:, :, :, k, :].transpose(
                0, 3, 2, 1
            )
        # xcont[(i16,s), (ic, b32)] = xc[b32, ic*16+i16, s] / NI
        xcont = np.ascontiguousarray(
            (xc.reshape(B_CORE, IC, I16, S) / NI)
            .transpose(2, 3, 1, 0)
            .reshape(P, IC * B_CORE)
            .astype(np.float16)
        )
        in_maps.append(
            {
                "xdiag": np.ascontiguousarray(xd.reshape(NGRP, P, IC * P)),
                "wpack": wpack,
                "xcont": xcont,
                "sdel32": sdel32,
                "sdel32b": sdel32b,
                "srep32": srep32,
                "ident": ident,
            }
        )
    return in_maps


def _unshard(vout: np.ndarray) -> np.ndarray:
    """Per-core vout [B_CORE, (w,h)] -> [B_CORE, H, WD]."""
    return vout.reshape(B_CORE, WD, H).transpose(0, 2, 1)


def kernel(x: np.ndarray, W: np.ndarray) -> np.ndarray:
    from concourse import bass_utils

    if "nc" not in _CACHE:
        _CACHE["nc"] = _build_program(debug=False)
    nc = _CACHE["nc"]
    in_maps = _host_inputs(x, W)
    res = bass_utils.run_bass_kernel_spmd(nc, in_maps, list(range(N_CORES)))
    outs = [_unshard(res.results[c]["vout"]) for c in range(N_CORES)]
    return np.concatenate(outs, axis=0).astype(np.float32)


# revision 18
# speedup vs baseline: 1.2700x; 1.2700x over previous
"""DigitCaps (capsule routing) Trainium2 kernel, v2 (no-maxshift design).

Self-contained: hardcodes shapes for
  x: [256, 32, 8, 6, 6] f32, W: [1, 10, 1152, 16, 8] f32 -> v: [256, 10, 16] f32

Sharding: pure data parallelism over batch, 32 batch items per core on 8
cores, processed as 4 octet groups per core.

Per-core layout: partition p = (i16, b8); u free dims ordered (ic=72,
w=16, h=10) with h innermost so the big DVE muls run in fp16 2x mode.
u = W@x from block-diag packed fp16 matmuls (K=128: lhsT = host-built
block-diag x tile, rhs = repacked W, w-major/h-minor free order).

v2 changes vs v1:
  - s0 = mean_i u_i computed directly as a dense K=9216 contraction
    (72 accumulating matmuls of xcont[ic] @ wpack[:, ic], out [32, 160]),
    not by streaming u through sdelta matmuls. xcont is pre-scaled 1/NI.
  - NO softmax max-shift. Iteration-1 logits are in [-4, 6] so exp fits
    f16 directly; iteration-2 logits reach ~36 so exp goes to bf16
    (range to 3e38) and pr = u*c is bf16. The softmax division happens in
    squash via the streamed denominator (PSUM-accumulated dps matmuls).
  - Unified [32]-partition s/d/squash layout: per-group sdelta32/srepl32
    selection matrices let all 4 groups accumulate into one PSUM tile;
    one reduce + one squash + one output DMA per iteration.
  - u PSUM->SBUF copies in chunks of 6 ic (fewer ACT instructions), some
    offloaded to the otherwise-idle gpsimd (Pool) engine.
  - a-pass fold over w either as in-place DVE fp16 fold tree or on PE
    (16 accumulating identity matmuls), chosen per group for balance.
"""

import numpy as np

# ---- problem constants (hardcoded) ----
B_FULL = 256
N_CORES = 8
B_CORE = B_FULL // N_CORES          # 32
NGRP = 4                            # octet groups per core
B8 = 8                              # batch per group
H = 10
WD = 16
WH = WD * H                         # 160
S = 8
NI = 1152
I16 = 16
IC = NI // I16                      # 72
ICQ = 9                             # ic per wpack DMA chunk
XDC = 18                            # ic per xdiag DMA chunk
CPY = 3                             # ic per u psum copy tile (1 PSUM bank)
CPYS = 3                            # ic per sps matmul
CKS = 36                            # ic per s-pass mul chunk
HIC = IC // 2                       # 36 (a-pass half)
P = 128

_CACHE = {}


def _build_program(debug: bool, dumps: bool = False):
    import concourse.bacc as bacc
    import concourse.bass as bass
    import concourse.tile as tile
    from concourse import mybir

    f32 = mybir.dt.float32
    f16 = mybir.dt.float16
    bf16 = mybir.dt.bfloat16
    AX = mybir.AxisListType
    AF = mybir.ActivationFunctionType

    if not getattr(bacc, "_digitcaps_act_pin", False):
        _orig_gat = bacc.get_activation_tables

        def _pinned_gat(arch):
            tables = dict(_orig_gat(arch))
            both = {mybir.ActivationFunctionType.Exp, mybir.ActivationFunctionType.Ln}
            for name in tables:
                if name != "natural_log_exp_and_others" and both & tables[name]:
                    tables[name] = tables[name] - both
            return tables

        bacc.get_activation_tables = _pinned_gat
        bacc._digitcaps_act_pin = True

    nc = bacc.Bacc(
        "TRN2", target_bir_lowering=False, debug=debug, enable_asserts=False
    )

    xd_d = nc.dram_tensor("xdiag", [NGRP, P, IC * P], f16, kind="ExternalInput")
    w_d = nc.dram_tensor("wpack", [P, IC * WH], f16, kind="ExternalInput")
    xc_d = nc.dram_tensor("xcont", [P, IC * B_CORE], f16, kind="ExternalInput")
    sd_d = nc.dram_tensor("sdel32", [P, NGRP * B_CORE], f16, kind="ExternalInput")
    sdb_d = nc.dram_tensor("sdel32b", [P, NGRP * B_CORE], bf16, kind="ExternalInput")
    sr_d = nc.dram_tensor("srep32", [B_CORE, NGRP * P], f16, kind="ExternalInput")
    id_d = nc.dram_tensor("ident", [P, P], f16, kind="ExternalInput")
    out_d = nc.dram_tensor("vout", [B_CORE, WH], f32, kind="ExternalOutput")
    if dumps:
        dbg_u = nc.dram_tensor("dbg_u", [P, IC * WH], f16, kind="ExternalOutput")
        dbg_l = nc.dram_tensor("dbg_l", [P, NGRP * IC * H], f16, kind="ExternalOutput")
        dbg_c = nc.dram_tensor("dbg_c", [P, NGRP * IC * H], f16, kind="ExternalOutput")
        dbg_s = nc.dram_tensor("dbg_s", [B_CORE, 170], f32, kind="ExternalOutput")
        dbg_s2 = nc.dram_tensor("dbg_s2", [B_CORE, 170], f32, kind="ExternalOutput")
        dbg_v = nc.dram_tensor("dbg_v", [B_CORE, WH], f16, kind="ExternalOutput")
        dbg_s0 = nc.dram_tensor("dbg_s0", [B_CORE, WH], f32, kind="ExternalOutput")

    with tile.TileContext(nc) as tc:
        with (
            tc.tile_pool(name="const", bufs=1) as const_pool,
            tc.tile_pool(name="wp", bufs=1) as wp_pool,
            tc.tile_pool(name="xd", bufs=2) as xd_pool,
            tc.tile_pool(name="u", bufs=4) as u_pool,
            tc.tile_pool(name="prs", bufs=2) as prs_pool,
            tc.tile_pool(name="pra", bufs=2) as pra_pool,
            tc.tile_pool(name="lg", bufs=1) as lg_pool,
            tc.tile_pool(name="cexp", bufs=1) as c_pool,
            tc.tile_pool(name="small", bufs=2) as small_pool,
            tc.tile_pool(name="vv", bufs=1) as vv_pool,
            tc.tile_pool(name="psum_u", bufs=2, space="PSUM") as psum_u,
            tc.tile_pool(name="psum_s", bufs=1, space="PSUM") as psum_s,
            tc.tile_pool(name="psum_v", bufs=2, space="PSUM") as psum_v,
        ):
            xcont = const_pool.tile([P, IC, B_CORE], f16, tag="xcont")
            nc.sync.dma_start(
                xcont[:], xc_d[:].rearrange("p (ic b) -> p ic b", ic=IC)
            )
            srep32 = const_pool.tile([B_CORE, NGRP, P], f16, tag="srep32")
            nc.sync.dma_start(
                srep32[:], sr_d[:].rearrange("b (g p) -> b g p", g=NGRP)
            )

            def load_xd(g, xc):
                xd = xd_pool.tile([P, XDC, P], f16, tag="xd")
                nc.sync.dma_start(
                    xd[:],
                    xd_d[g].rearrange("p (ic m) -> p ic m", ic=IC)[
                        :, xc : xc + XDC
                    ],
                )
                return xd

            # resident W pack, split per chunk so deps are chunk-granular.
            # Emission interleaves group 0's xd loads between wpack chunks and
            # streams the 72 s0 matmuls as each W chunk arrives.
            wpq = [None] * (IC // ICQ)
            xds0 = []

            def load_wp(qi):
                wq = wp_pool.tile([P, ICQ, WH], f16, tag=f"wp{qi}")
                nc.sync.dma_start(
                    wq[:],
                    w_d[:].rearrange("p (ic f) -> p ic f", f=WH)[
                        :, qi * ICQ : (qi + 1) * ICQ
                    ],
                )
                wpq[qi] = wq

            # s0[b, (w,h)] = sum_{ic,i16,s} x[b,i,s]/NI * W[h,i,w,s]
            s0ps_t = psum_s.tile([B_CORE, CPYS, 170], f32, tag="sps")
            s0ps = s0ps_t[:, 0, 0:WH]

            def s0_chunk(qi):
                for t in range(ICQ):
                    ic = qi * ICQ + t
                    nc.tensor.matmul(
                        s0ps,
                        xcont[:, ic, :],
                        wpq[qi][:, t, :],
                        start=(ic == 0),
                        stop=(ic == IC - 1),
                    )

            load_wp(0)
            xds0.append(load_xd(0, 0))
            s0_chunk(0)
            load_wp(1)
            s0_chunk(1)
            load_wp(2)
            xds0.append(load_xd(0, XDC))
            s0_chunk(2)
            load_wp(3)
            s0_chunk(3)
            load_wp(4)
            xds0.append(load_xd(0, 2 * XDC))
            s0_chunk(4)
            load_wp(5)
            s0_chunk(5)
            load_wp(6)
            xds0.append(load_xd(0, 3 * XDC))
            s0_chunk(6)
            load_wp(7)
            s0_chunk(7)

            # remaining consts (needed only from iteration 1 onward)
            sdel32 = const_pool.tile([P, NGRP, B_CORE], f16, tag="sdel32")
            nc.sync.dma_start(
                sdel32[:], sd_d[:].rearrange("p (g b) -> p g b", g=NGRP)
            )
            sdel32b = const_pool.tile([P, NGRP, B_CORE], bf16, tag="sdel32b")
            nc.sync.dma_start(
                sdel32b[:], sdb_d[:].rearrange("p (g b) -> p g b", g=NGRP)
            )
            ident = const_pool.tile([P, P], f16, tag="ident")
            nc.sync.dma_start(ident[:], id_d[:])

            # persistent logits, one tile per group so iteration-(k+1)'s
            # exp(g) only waits on group g's a-pass (tile-granular deps)
            logits = [
                lg_pool.tile([P, IC, H], f16, tag=f"logits{g}", name=f"logits{g}")
                for g in range(NGRP)
            ]

            # V = running sum of v (f16) on 32 partitions; sun = s/d scratch
            V32 = vv_pool.tile([B_CORE, WD, H], f16, tag="V32")
            vb16 = [
                vv_pool.tile([P, WD, H], f16, tag=f"vb16_{g}", name=f"vb16_{g}")
                for g in range(NGRP)
            ]
            sun32 = vv_pool.tile([B_CORE, 170], f32, tag="sun32")

            us = []

            def ugen_part(g, uh, xchunks):
                """u-gen for group g over the given xd chunk indices.
                uh = (ua, ub) half tiles of 36 ic each -- separate tiles so
                consumers of one half don't wait on the other (tile-granular
                dependency tracking). PSUM tile is [P, 2, 512] f32 (two
                banks; each matmul's 160-col slice stays within one bank) so
                one ACT instruction drains 6 ic at a time."""
                for xi in xchunks:
                    xc = xi * XDC
                    xd = xds0[xi] if g == 0 else load_xd(g, xc)
                    for j in range(0, XDC, 2 * CPY):
                        ps = psum_u.tile([P, 2, 512], f32, tag="ups")
                        for t in range(2 * CPY):
                            ic = xc + j + t
                            k, m = divmod(t, CPY)
                            nc.tensor.matmul(
                                ps[:, k, m * WH : (m + 1) * WH],
                                xd[:, j + t, :],
                                wpq[ic // ICQ][:, ic % ICQ, :],
                                start=True,
                                stop=True,
                            )
                        ic0 = xc + j
                        u = uh[ic0 // HIC]
                        nc.scalar.copy(
                            u[:, (ic0 % HIC) : (ic0 % HIC) + 2 * CPY].rearrange(
                                "p (k a) w h -> p k a w h", k=2
                            ),
                            ps[:, :, 0 : CPY * WH].rearrange(
                                "p k (a w h) -> p k a w h", w=WD, h=H
                            ),
                        )

            def ugen(g):
                uh = (
                    u_pool.tile([P, HIC, WD, H], f16, tag="ua", name="ua"),
                    u_pool.tile([P, HIC, WD, H], f16, tag="ub", name="ub"),
                )
                ugen_part(g, uh, range(IC // XDC))
                us.append(uh)

            def squash(it):
                """sun32 -> v; updates V32 (it<2) or returns vfin (it=2)."""
                sw = sun32[:, 0:WH].rearrange("b (w h) -> b w h", h=H)
                if it == 0:
                    s = sw
                else:
                    dinv = small_pool.tile([B_CORE, H], f32, tag="dinv")
                    nc.vector.reciprocal(dinv[:], sun32[:, WH:170])
                    st = small_pool.tile([B_CORE, WD, H], f32, tag="st")
                    nc.vector.tensor_mul(
                        st[:], sw, dinv[:].unsqueeze(1).to_broadcast([B_CORE, WD, H])
                    )
                    s = st[:]
                s2 = small_pool.tile([B_CORE, WD, H], f32, tag="s2")
                nc.scalar.activation(s2[:], s, AF.Square)
                sq = small_pool.tile([B_CORE, H], f32, tag="sq")
                nc.vector.reduce_sum(
                    sq[:], s2[:].rearrange("b w h -> b h w"), axis=AX.X
                )
                lgq = small_pool.tile([B_CORE, H], f32, tag="lgq")
                nc.scalar.activation(lgq[:], sq[:], AF.Ln)
                rt = small_pool.tile([B_CORE, H], f32, tag="rt")
                nc.scalar.activation(rt[:], lgq[:], AF.Exp, scale=0.5)
                onep = small_pool.tile([B_CORE, H], f32, tag="onep")
                nc.vector.tensor_scalar_add(onep[:], sq[:], 1.0)
                rr = small_pool.tile([B_CORE, H], f32, tag="rr")
                nc.vector.reciprocal(rr[:], onep[:])
                f = small_pool.tile([B_CORE, H], f32, tag="f")
                nc.vector.tensor_mul(f[:], rt[:], rr[:])
                fb = f[:].unsqueeze(1).to_broadcast([B_CORE, WD, H])
                if it == 2:
                    vfin = small_pool.tile([B_CORE, WD, H], f32, tag="vfin")
                    nc.vector.tensor_mul(vfin[:], s, fb)
                    return vfin
                if it == 0:
                    nc.vector.tensor_mul(V32[:], s, fb)
                else:
                    v16 = small_pool.tile([B_CORE, WD, H], f16, tag="v16")
                    nc.vector.tensor_mul(v16[:], s, fb)
                    nc.vector.tensor_add(V32[:], V32[:], v16[:])
                return None

            def vbcast():
                """vb16[:, g] = broadcast of V32 rows g*8..g*8+8."""
                for g in range(NGRP):
                    pv = psum_v.tile([P, HIC, H], f32, tag="pv")
                    vbp = pv[:, 0:WD, :].rearrange("p w h -> p w h")
                    nc.tensor.matmul(
                        vbp, srep32[:, g, :], V32[:], start=True, stop=True
                    )
                    nc.scalar.copy(vb16[g][:], vbp)

            def apass(g, pe_fold=False):
                """logits[g] = sum_w u * vb16[g], in two ic-halves."""
                for hi, a in enumerate((0, HIC)):
                    u = us[g][hi]
                    pra = pra_pool.tile([P, HIC, WD, H], f16, tag="pra")
                    vbb = vb16[g][:].unsqueeze(1).to_broadcast([P, HIC, WD, H])
                    nc.vector.tensor_mul(pra[:], u[:], vbb)
                    lslice = logits[g][:, a : a + HIC, :]
                    if pe_fold:
                        pa = psum_v.tile([P, HIC, H], f32, tag="pv")
                        for w in range(WD):
                            nc.tensor.matmul(
                                pa[:],
                                ident[:],
                                pra[:, :, w, :],
                                start=(w == 0),
                                stop=(w == WD - 1),
                            )
                        nc.scalar.copy(lslice, pa[:])
                    else:
                        nc.vector.tensor_add(
                            pra[:, :, 0:8, :],
                            pra[:, :, 0:8, :],
                            pra[:, :, 8:16, :],
                        )
                        nc.vector.tensor_add(
                            pra[:, :, 0:4, :],
                            pra[:, :, 0:4, :],
                            pra[:, :, 4:8, :],
                        )
                        nc.vector.tensor_add(
                            pra[:, :, 0:2, :],
                            pra[:, :, 0:2, :],
                            pra[:, :, 2:4, :],
                        )
                        nc.vector.tensor_add(
                            lslice, pra[:, :, 0, :], pra[:, :, 1, :]
                        )

            def spass(g, cexp_g, prdt, sdel, sps32, dps32):
                """stream d (dps) and s (sps) partial sums for group g."""
                # d: 2 matmuls over cexp halves
                for hi, a in enumerate((0, HIC)):
                    nc.tensor.matmul(
                        dps32[:],
                        sdel[:, g, :],
                        cexp_g[:, a : a + HIC, :],
                        start=(g == 0 and hi == 0),
                        stop=(g == NGRP - 1 and hi == 1),
                    )
                for hi, c0 in enumerate(range(0, IC, CKS)):
                    u = us[g][hi]
                    pr = prs_pool.tile([P, CKS, WD, H], prdt, tag="pr")
                    cb = (
                        cexp_g[:, c0 : c0 + CKS, :]
                        .unsqueeze(2)
                        .to_broadcast([P, CKS, WD, H])
                    )
                    nc.vector.tensor_mul(pr[:], u[:], cb)
                    for j in range(0, CKS, CPYS):
                        ic = c0 + j
                        nc.tensor.matmul(
                            sps32[:, :, 0:WH],
                            sdel[:, g, :],
                            pr[:, j : j + CPYS],
                            start=(g == 0 and ic == 0),
                            stop=(g == NGRP - 1 and ic == IC - CPYS),
                        )

            def s_reduce(sps32, dps32):
                nc.vector.reduce_sum(
                    sun32[:, 0:WH],
                    sps32[:, :, 0:WH].rearrange("b a f -> b f a"),
                    axis=AX.X,
                )
                nc.vector.reduce_sum(
                    sun32[:, WH:170],
                    dps32[:].rearrange("b i h -> b h i"),
                    axis=AX.X,
                )

            # ======== iteration 0: s0 -> v0 -> vbcast -> apass ========
            # squash(0) first in emission so its ACT/DVE ops aren't queued
            # behind group-0's u copies; ugen(0) keeps PE busy meanwhile.
            # vbcast slots between ugen(0) halves so the PE in-order queue
            # reaches it as soon as V32 is ready (not after all of ugen).
            nc.scalar.copy(sun32[:, 0:WH], s0ps)
            if dumps:
                nc.sync.dma_start(dbg_s0[:], s0ps)
            squash(0)
            uh0 = (
                u_pool.tile([P, HIC, WD, H], f16, tag="ua", name="u0a"),
                u_pool.tile([P, HIC, WD, H], f16, tag="ub", name="u0b"),
            )
            ugen_part(0, uh0, range(2))
            vbcast()
            ugen_part(0, uh0, range(2, IC // XDC))
            us.append(uh0)
            apass(0, pe_fold=False)
            for g in range(1, NGRP):
                ugen(g)
                apass(g, pe_fold=(g >= 2))

            if dumps:
                nc.sync.dma_start(
                    dbg_u[:, 0 : HIC * WH],
                    us[0][0][:].rearrange("p ic w h -> p (ic w h)"),
                )
                nc.sync.dma_start(
                    dbg_u[:, HIC * WH :],
                    us[0][1][:].rearrange("p ic w h -> p (ic w h)"),
                )
                for g in range(NGRP):
                    nc.sync.dma_start(
                        dbg_l[:, g * IC * H : (g + 1) * IC * H],
                        logits[g][:].rearrange("p ic h -> p (ic h)"),
                    )

            # ======== iteration 1 (f16 exp) ========
            cexp1 = [
                c_pool.tile([P, IC, H], f16, tag=f"cexp1_{g}", name=f"cexp1_{g}")
                for g in range(NGRP)
            ]
            dps32 = psum_s.tile([B_CORE, HIC, H], f32, tag="dps")
            sps32 = psum_s.tile([B_CORE, CPYS, 170], f32, tag="sps")
            for g in range(NGRP):
                nc.scalar.activation(cexp1[g][:], logits[g][:], AF.Exp)
                spass(g, cexp1[g], f16, sdel32, sps32, dps32)
            if dumps:
                for g in range(NGRP):
                    nc.sync.dma_start(
                        dbg_c[:, g * IC * H : (g + 1) * IC * H],
                        cexp1[g][:].rearrange("p ic h -> p (ic h)"),
                    )
            s_reduce(sps32, dps32)
            if dumps:
                nc.sync.dma_start(dbg_s[:], sun32[:])
            squash(1)
            if dumps:
                nc.sync.dma_start(
                    dbg_v[:], V32[:].rearrange("b w h -> b (w h)")
                )
            vbcast()
            for g in range(NGRP):
                apass(g, pe_fold=(g < NGRP - 1))

            # ======== iteration 2 (bf16 exp, bf16 pr) ========
            cexp2 = [
                c_pool.tile([P, IC, H], bf16, tag=f"cexp2_{g}", name=f"cexp2_{g}")
                for g in range(NGRP)
            ]
            dps32b = psum_s.tile([B_CORE, HIC, H], f32, tag="dps")
            sps32b = psum_s.tile([B_CORE, CPYS, 170], f32, tag="sps")
            for g in range(NGRP):
                nc.scalar.activation(cexp2[g][:], logits[g][:], AF.Exp)
                spass(g, cexp2[g], bf16, sdel32b, sps32b, dps32b)
            s_reduce(sps32b, dps32b)
            if dumps:
                nc.sync.dma_start(dbg_s2[:], sun32[:])
            vfin = squash(2)
            nc.sync.dma_start(
                out_d[:], vfin[:].rearrange("b w h -> b (w h)")
            )

    nc.compile()
    return nc


def _host_inputs(x: np.ndarray, W: np.ndarray):
    """Build per-core input maps."""
    xr = np.ascontiguousarray(x.reshape(B_FULL, NI, S).astype(np.float32, copy=False))
    W0 = np.asarray(W, dtype=np.float32).reshape(H, NI, WD, S)
    # wpack[(i16,s), (ic, w, h)] = W0[h, ic*16+i16, w, s]
    wpack = np.ascontiguousarray(
        W0.reshape(H, IC, I16, WD, S)
        .transpose(2, 4, 1, 3, 0)
        .reshape(P, IC * WH)
        .astype(np.float16)
    )
    # sdel32[(i16,b8), (g, b32)] = (b32 == g*8 + b8)
    b8 = np.arange(P) % B8
    g_idx = np.arange(NGRP)
    b32 = np.arange(B_CORE)
    sdel = (
        b32[None, None, :] == (g_idx[None, :, None] * B8 + b8[:, None, None])
    ).astype(np.float16)
    sdel32 = np.ascontiguousarray(sdel.reshape(P, NGRP * B_CORE))
    import ml_dtypes

    sdel32b = sdel32.astype(ml_dtypes.bfloat16)
    # srep32[b32, (g, p)] = (b32 == g*8 + p%8)
    srep = (
        b32[:, None, None] == (g_idx[None, :, None] * B8 + b8[None, None, :])
    ).astype(np.float16)
    srep32 = np.ascontiguousarray(srep.reshape(B_CORE, NGRP * P))
    ident = np.eye(P, dtype=np.float16)

    in_maps = []
    for c in range(N_CORES):
        xc = xr[c * B_CORE : (c + 1) * B_CORE]  # [32, 1152, 8]
        # xdiag[g, (i16,s), ic*128 + i16*8 + b] = xc[g*8+b, ic*16+i16, s]
        xd = np.zeros((NGRP, P, IC, I16, B8), dtype=np.float16)
        xg = xc.reshape(NGRP, B8, IC, I16, S).astype(np.float16)
        for k in range(I16):
            xd[:, k * S : (k + 1) * S, :, k, :] = xg[:, :, :, k, :].transpose(
                0, 3, 2, 1
            )
        # xcont[(i16,s), (ic, b32)] = xc[b32, ic*16+i16, s] / NI
        xcont = np.ascontiguousarray(
            (xc.reshape(B_CORE, IC, I16, S) / NI)
            .transpose(2, 3, 1, 0)
            .reshape(P, IC * B_CORE)
            .astype(np.float16)
        )
        in_maps.append(
            {
                "xdiag": np.ascontiguousarray(xd.reshape(NGRP, P, IC * P)),
                "wpack": wpack,
                "xcont": xcont,
                "sdel32": sdel32,
                "sdel32b": sdel32b,
                "srep32": srep32,
                "ident": ident,
            }
        )
    return in_maps


def _unshard(vout: np.ndarray) -> np.ndarray:
    """Per-core vout [B_CORE, (w,h)] -> [B_CORE, H, WD]."""
    return vout.reshape(B_CORE, WD, H).transpose(0, 2, 1)


def kernel(x: np.ndarray, W: np.ndarray) -> np.ndarray:
    from concourse import bass_utils

    if "nc" not in _CACHE:
        _CACHE["nc"] = _build_program(debug=False)
    nc = _CACHE["nc"]
    in_maps = _host_inputs(x, W)
    res = bass_utils.run_bass_kernel_spmd(nc, in_maps, list(range(N_CORES)))
    outs = [_unshard(res.results[c]["vout"]) for c in range(N_CORES)]
    return np.concatenate(outs, axis=0).astype(np.float32)
